# revision 1
# baseline (speedup 1.0000x reference)
"""Trainium2 Bass kernel for nn_BiLSTM_M_61615600828569 (segment_reduce).

Full computation per batch:
  span_emb = masked-max-pool of token windows   (B,256,768)
  vertex_emb = masked-mean over coref spans     (B,128,768)
  head/tail  = vertex gather by relation        (B,512,768)
  feat = [head, eh, tail, et, head*tail]        (B,512,2344)
  out  = relu(feat @ W1) @ W2 + b2              (B,512,97)

Sharding: data-parallel over batch; 16 batches / 8 cores = 2 per core.
All index work (gather tables, one-hot select matrices, pooling weights)
is precomputed on host; all float math runs on device in bf16 with fp32
PSUM accumulation, in transposed layout (features on partitions) so the
final predict.T has the 97 classes on partitions for a per-partition
bias add.

Span pooling: token rows are fetched with dma_gather at QUAD granularity
(elem = 4 overlapping rows via elem_step) — pass 0 reads rows start..start+3
(rows past the width are killed by one broadcast additive -2e30 mask on the
DVE); pass 1 reads start+min(4, w-3).. which with pass 0 covers
[start, start+w] for w>=3, and is redirected to a staged NEG quad for w<3.
Passes are split per batch so batch-0 compute starts while batch-1 still
gathers.
W1 is zero-padded to 20 uniform 128-row contraction chunks so the eh/et
blocks ride the same accumulation loop (their rhs rows past row 19 are
zeros times zero weights).
"""
import numpy as np
import ml_dtypes
from contextlib import ExitStack

import concourse.bass as bass
import concourse.bacc as bacc
import concourse.tile as tile
from concourse import mybir
from concourse import bass_utils

BF16 = ml_dtypes.bfloat16

B, S, D = 16, 1024, 768
NS, MAXW = 256, 8
V, C = 128, 6
R = 512
REL, HID, DIS = 97, 384, 20
NEG = -1e30

NCORES = 8
NB = B // NCORES          # batches per core = 2
GS = NB * NS              # spans per core = 512
NQ = GS // 128            # span groups = 4
NPASS = 2                 # quad passes per batch
SENT_ROWS = NB * S + 4    # staged sentence rows + four NEG rows (NEG quad)
NEGROW = NB * S
NKC = 20                  # uniform 128-row W1 contraction chunks
W1PAD = NKC * 128

FEAT_BLOCKS = [(0, 768), (768, 788), (788, 1556), (1556, 1576), (1576, 2344)]


def _patch_drain_and_barrier():
    """Walrus rejects >1 explicit sync wait on a Drain (TPB_CTRL), but Tile's
    tail drain waits on every used proc sem at once. Emit one single-wait
    drain per proc instead; the final drain then needs no waits."""
    import concourse.tile as tile_mod
    from concourse.vector_clock import VectorClock, ScopedClock

    if getattr(tile_mod.TileContext, "_ant_drain_patched", False):
        return

    def _patched(self, tick_clock, wait_clock):
        full = tick_clock.global_clock
        n = len(full)
        engines = [self.nc.sync, self.nc.vector, self.nc.scalar,
                   self.nc.tensor, self.nc.gpsimd]
        for i, p in enumerate([q for q in range(n) if full[q] > 0]):
            vec = [full[q] if q == p else 0 for q in range(n)]
            d = engines[i % len(engines)].drain()
            wait_clock.add_sem_waits(d.ins, ScopedClock({None: VectorClock(vec)}))
        self.nc.sync.drain()
        self.nc.all_engine_barrier()
        popped = self.nc._tile_sem_poison_stack.pop()
        assert popped is self._sem_poison
        self.nc.clear_and_free_semaphores(list(self.sems.allocated().values()))
        self.nc.all_engine_barrier()

    tile_mod.TileContext._drain_and_barrier = _patched
    tile_mod.TileContext._ant_drain_patched = True


_patch_drain_and_barrier()

_NC_CACHE = None


def _build():
    """One-core program; SPMD-replicated across the 8 cores."""
    bf = mybir.dt.bfloat16
    f32 = mybir.dt.float32
    AF = mybir.ActivationFunctionType
    MAX = mybir.AluOpType.max

    nc = bacc.Bacc("TRN2", target_bir_lowering=False, debug=False, num_devices=1)

    sent = nc.dram_tensor("sent", (SENT_ROWS, D), bf, kind="ExternalInput")
    gidx = nc.dram_tensor("gidx", (128, NB, NPASS, 16), mybir.dt.int16, kind="ExternalInput")
    w0m = nc.dram_tensor("w0m", (128, NQ, 3), f32, kind="ExternalInput")
    poolt = nc.dram_tensor("poolt", (128, NB, 2, V), bf, kind="ExternalInput")
    invcnt = nc.dram_tensor("invcnt", (V, NB), f32, kind="ExternalInput")
    hsel = nc.dram_tensor("hsel", (V, NB, R), bf, kind="ExternalInput")
    tsel = nc.dram_tensor("tsel", (V, NB, R), bf, kind="ExternalInput")
    dist = nc.dram_tensor("dist", (DIS, DIS), bf, kind="ExternalInput")
    ehsel = nc.dram_tensor("ehsel", (DIS, NB, R), bf, kind="ExternalInput")
    etsel = nc.dram_tensor("etsel", (DIS, NB, R), bf, kind="ExternalInput")
    w1 = nc.dram_tensor("w1", (128, NKC, HID), bf, kind="ExternalInput")
    w2 = nc.dram_tensor("w2", (128, HID // 128, REL), bf, kind="ExternalInput")
    b2t = nc.dram_tensor("b2t", (REL, 1), f32, kind="ExternalInput")
    outd = nc.dram_tensor("outd", (128, NB, R), f32, kind="ExternalOutput")

    # overlapping-quad view of the staged sentence: row i -> rows [i, i+3]
    sent_quads = bass.AP(tensor=sent.ap().tensor, offset=0,
                         ap=[[D, SENT_ROWS - 3], [1, 4 * D]])

    with tile.TileContext(nc) as tc, ExitStack() as ctx:
        consts = ctx.enter_context(tc.tile_pool(name="consts", bufs=1))
        work = ctx.enter_context(tc.tile_pool(name="work", bufs=1))
        perb = ctx.enter_context(tc.tile_pool(name="perb", bufs=2))
        psums = ctx.enter_context(tc.tile_pool(name="psums", bufs=1, space="PSUM"))

        def psum_tile(name, tag, bufs):
            return psums.tile([128, R], mybir.dt.float32, space="PSUM",
                              tag=tag, bufs=bufs, name=name)

        # ---- gather index table first: the Q7 is the gather's serial resource ----
        idx_t = consts.tile([128, NB, NPASS, 16], mybir.dt.int16)
        nc.sync.dma_start(out=idx_t[:], in_=gidx.ap())
        w0m_t = consts.tile([128, NQ, 3], f32)
        nc.sync.dma_start(out=w0m_t[:], in_=w0m.ap())

        # ---- quad gathers: per batch h, 2 passes of 256 quad-descriptors ----
        pair_tiles = [[None] * NPASS for _ in range(NB)]
        for h in range(NB):
            for j in range(NPASS):
                gt = work.tile([128, 2, 4 * D], bf, name=f"gp_{h}_{j}", tag=f"gp_{h}_{j}")
                nc.gpsimd.dma_gather(
                    out_ap=gt[:],
                    in_ap=sent_quads,
                    idxs_ap=idx_t[:, h, j, :],
                    num_idxs=256,
                    num_idxs_reg=256,
                    elem_size=4 * D,
                    elem_step=D,
                    single_packet=False,
                )
                pair_tiles[h][j] = gt

        # ---- constant loads (one DMA each) ----
        w1_t = consts.tile([128, NKC, HID], bf)
        nc.sync.dma_start(out=w1_t[:], in_=w1.ap())
        w2_t = consts.tile([128, HID // 128, REL], bf)
        nc.sync.dma_start(out=w2_t[:], in_=w2.ap())
        b2_t = consts.tile([REL, 1], f32)
        nc.sync.dma_start(out=b2_t[:], in_=b2t.ap())
        inv_t = consts.tile([V, NB], f32)
        nc.sync.dma_start(out=inv_t[:], in_=invcnt.ap())
        pt_t = consts.tile([128, NB, 2, V], bf)
        nc.sync.dma_start(out=pt_t[:], in_=poolt.ap())
        hs_t = consts.tile([V, NB, R], bf)
        nc.sync.dma_start(out=hs_t[:], in_=hsel.ap())
        ts_t = consts.tile([V, NB, R], bf)
        nc.sync.dma_start(out=ts_t[:], in_=tsel.ap())
        dist_t = consts.tile([DIS, DIS], bf)
        nc.sync.dma_start(out=dist_t[:], in_=dist.ap())
        ehs_t = consts.tile([DIS, NB, R], bf)
        nc.sync.dma_start(out=ehs_t[:], in_=ehsel.ap())
        ets_t = consts.tile([DIS, NB, R], bf)
        nc.sync.dma_start(out=ets_t[:], in_=etsel.ap())

        # ---- max-tree per batch: quads -> span_emb q-slices ----
        # quad-0 rows 1..3 are folded in with scalar_tensor_tensor:
        # acc = max(acc, row_r + mask_r), mask_r = -2e30 where r > width
        sem_b = []  # sem_b[h][p, cc, :] = span_emb[(2h+cc)*128 + p]
        for h in range(NB):
            g0 = pair_tiles[h][0][:].rearrange("p q (r d) -> p q r d", r=4)
            g1 = pair_tiles[h][1][:].rearrange("p q (r d) -> p q r d", r=4)
            q0m = work.tile([128, 2, D], bf, name=f"q0m_{h}", tag=f"q0m_{h}")
            for q in range(2):
                gq = 2 * h + q
                nc.vector.scalar_tensor_tensor(
                    out=q0m[:, q, :], in0=g0[:, q, 1, :], scalar=w0m_t[:, gq, 0:1],
                    in1=g0[:, q, 0, :], op0=mybir.AluOpType.add, op1=MAX)
                nc.vector.scalar_tensor_tensor(
                    out=q0m[:, q, :], in0=g0[:, q, 2, :], scalar=w0m_t[:, gq, 1:2],
                    in1=q0m[:, q, :], op0=mybir.AluOpType.add, op1=MAX)
                nc.vector.scalar_tensor_tensor(
                    out=q0m[:, q, :], in0=g0[:, q, 3, :], scalar=w0m_t[:, gq, 2:3],
                    in1=q0m[:, q, :], op0=mybir.AluOpType.add, op1=MAX)
            m2a = work.tile([128, 2, D], bf, name=f"m2a_{h}", tag=f"m2a_{h}")
            nc.vector.tensor_tensor(out=m2a[:], in0=g1[:, :, 0, :], in1=g1[:, :, 1, :], op=MAX)
            m2b = work.tile([128, 2, D], bf, name=f"m2b_{h}", tag=f"m2b_{h}")
            nc.vector.tensor_tensor(out=m2b[:], in0=g1[:, :, 2, :], in1=g1[:, :, 3, :], op=MAX)
            nc.vector.tensor_tensor(out=m2a[:], in0=m2a[:], in1=m2b[:], op=MAX)
            sh = work.tile([128, 2, D], bf, name=f"sem_{h}", tag=f"sem_{h}")
            nc.vector.tensor_tensor(out=sh[:], in0=q0m[:], in1=m2a[:], op=MAX)
            sem_b.append(sh)

        # ---- Ew = dis_embed @ W1-block, shared by both batches ----
        ew_sbs = {}
        for name, ci in (("ewb", 6), ("ewd", 13)):
            ps_e = psums.tile([DIS, HID], mybir.dt.float32, space="PSUM",
                              tag="out", bufs=1, name=f"ps_{name}")
            nc.tensor.matmul(ps_e[:], lhsT=dist_t[:], rhs=w1_t[:DIS, ci, :],
                             start=True, stop=True)
            ew_sb = consts.tile([DIS, HID], bf, name=f"{name}_sb")
            nc.scalar.activation(ew_sb[:], ps_e[:], AF.Copy)
            ew_sbs[name] = ew_sb

        # ---- per-batch compute, batch-interleaved so the PE stays fed ----
        v_sbs, vw_sbs, head_ts, tail_ts, prod_ts, hid_ts = {}, {}, {}, {}, {}, {}
        for b in range(NB):
            ps_v = psums.tile([128, D], mybir.dt.float32, space="PSUM",
                              tag="ps_v", bufs=1, name="ps_v")
            for cc in range(2):
                for n0, nsz in ((0, 512), (512, 256)):
                    nc.tensor.matmul(
                        ps_v[:, n0 : n0 + nsz],
                        lhsT=pt_t[:, b, cc, :],
                        rhs=sem_b[b][:, cc, n0 : n0 + nsz],
                        start=(cc == 0),
                        stop=(cc == 1),
                    )
            v_sb = perb.tile([V, D], bf, tag="v_sb")
            nc.scalar.activation(v_sb[:], ps_v[:], AF.Copy, scale=inv_t[:, b : b + 1])
            v_sbs[b] = v_sb

            # V_emb.T chunks (for Vw), then Vw_a/Vw_c = (V_emb @ W1a/c) * inv
            vt_sb = perb.tile([128, 6, V], bf, tag="vt_sb")
            for m in range(6):
                ps_vt = psum_tile(f"ps_vt", "sel", 3)
                for cc in range(2):
                    nc.tensor.matmul(ps_vt[:, :V],
                                     lhsT=sem_b[b][:, cc, m * 128 : (m + 1) * 128],
                                     rhs=pt_t[:, b, cc, :],
                                     start=(cc == 0), stop=(cc == 1))
                nc.any.tensor_copy(vt_sb[:, m, :], ps_vt[:, :V])
            vw_a = perb.tile([V, HID], bf, tag="vw_a")
            vw_c = perb.tile([V, HID], bf, tag="vw_c")
            for name, vw, c0 in (("a", vw_a, 0), ("c", vw_c, 7)):
                ps_vw = psum_tile(f"ps_vw", "sel", 3)
                for m in range(6):
                    nc.tensor.matmul(ps_vw[:, :HID], lhsT=vt_sb[:, m, :],
                                     rhs=w1_t[:, c0 + m, :],
                                     start=(m == 0), stop=(m == 5))
                nc.scalar.activation(vw[:], ps_vw[:, :HID], AF.Copy,
                                     scale=inv_t[:, b : b + 1])
            vw_sbs[b] = (vw_a, vw_c)

        for b in range(NB):
            head_t = perb.tile([128, 6, R], bf, tag="head_t")
            tail_t = perb.tile([128, 6, R], bf, tag="tail_t")
            prod_t = perb.tile([128, 6, R], bf, tag="prod_t")
            for m in range(6):
                ps_h = psum_tile("ps_h", "sel", 3)
                nc.tensor.matmul(ps_h[:], lhsT=v_sbs[b][:, m * 128 : (m + 1) * 128],
                                 rhs=hs_t[:, b, :], start=True, stop=True)
                nc.any.tensor_copy(head_t[:, m, :], ps_h[:])
                ps_t2 = psum_tile("ps_t2", "sel", 3)
                nc.tensor.matmul(ps_t2[:], lhsT=v_sbs[b][:, m * 128 : (m + 1) * 128],
                                 rhs=ts_t[:, b, :], start=True, stop=True)
                nc.any.tensor_copy(tail_t[:, m, :], ps_t2[:])
                nc.vector.tensor_tensor(out=prod_t[:, m, :], in0=head_t[:, m, :],
                                        in1=tail_t[:, m, :], op=mybir.AluOpType.mult)
            head_ts[b], tail_ts[b], prod_ts[b] = head_t, tail_t, prod_t

        for b in range(NB):
            vw_a, vw_c = vw_sbs[b]
            hid_t = perb.tile([128, 3, R], bf, tag="hid_t")
            for m3 in range(3):
                msl = slice(m3 * 128, (m3 + 1) * 128)
                chunks = [(vw_a[:, msl], hs_t[:, b, :]),
                          (ew_sbs["ewb"][:, msl], ehs_t[:, b, :]),
                          (vw_c[:, msl], ts_t[:, b, :]),
                          (ew_sbs["ewd"][:, msl], ets_t[:, b, :])]
                chunks += [(w1_t[:, 14 + m, msl], prod_ts[b][:, m, :]) for m in range(6)]
                ps_hid = psum_tile("ps_hid", "hid", 2)
                for i, (lhsT, rhs_ap) in enumerate(chunks):
                    nc.tensor.matmul(ps_hid[:], lhsT=lhsT, rhs=rhs_ap,
                                     start=(i == 0), stop=(i == len(chunks) - 1))
                nc.scalar.activation(hid_t[:, m3, :], ps_hid[:], AF.Relu)
            hid_ts[b] = hid_t

        out_sb = work.tile([128, NB, R], f32)
        for b in range(NB):
            ps_o = psum_tile("ps_o", "out", 1)
            for kc in range(3):
                nc.tensor.matmul(
                    ps_o[:REL, :], lhsT=w2_t[:, kc, :], rhs=hid_ts[b][:, kc, :],
                    start=(kc == 0), stop=(kc == 2),
                )
            nc.scalar.activation(out_sb[:REL, b, :], ps_o[:REL, :], AF.Identity, bias=b2_t[:, 0:1])
        nc.sync.dma_start(out=outd.ap(), in_=out_sb[:])

    nc.compile()
    return nc


def _prep_core(c, sentence_repr, esi, vidx, vmask, ht, dis_h, dis_t,
               dis_embed_b, w1_p, w2_p, b2_f):
    """Build the per-core input map for batches [c*NB, c*NB+NB)."""
    bs = range(c * NB, c * NB + NB)

    sent = np.empty((SENT_ROWS, D), dtype=BF16)
    for j, b in enumerate(bs):
        sent[j * S : (j + 1) * S] = sentence_repr[b].astype(BF16)
    sent[NEGROW:] = BF16(NEG)

    # quad-gather tables: batch h, pass j, span i = q*128+p (local);
    # pass 0 base = start (rows past w masked); pass 1 base = start+min(4, w-3),
    # valid only for w>=3 (else NEG quad)
    starts = np.stack([esi[b, :, 0] for b in bs])                 # (NB, NS)
    widths = np.stack([esi[b, :, 1] - esi[b, :, 0] for b in bs])  # (NB, NS)
    gidx = np.empty((128, NB, NPASS, 16), dtype=np.int16)
    for h in range(NB):
        st, w = starts[h], widths[h]
        for j in range(NPASS):
            if j == 0:
                idx = st + h * S
            else:
                idx = np.where(w >= 3, st + np.minimum(4, w - 3) + h * S, NEGROW)
            flat = idx.astype(np.int16)                           # i = q*128+p order
            gidx[:, h, j, :] = np.tile(flat.reshape(-1, 16).T, (8, 1))

    # additive mask for quad-0 rows 1..3: -2e30 where row r > width
    w0mv = np.zeros((128, NQ, 3), dtype=np.float32)
    wq = widths.reshape(NQ, 128).T                                 # [p, q]
    for r in (1, 2, 3):
        w0mv[:, :, r - 1] = np.where(wq < r, np.float32(-2e30), np.float32(0.0))

    poolt = np.zeros((128, NB, 2, V), dtype=BF16)
    invcnt = np.zeros((V, NB), dtype=np.float32)
    hsel = np.zeros((V, NB, R), dtype=BF16)
    tsel = np.zeros((V, NB, R), dtype=BF16)
    ehsel = np.zeros((DIS, NB, R), dtype=BF16)
    etsel = np.zeros((DIS, NB, R), dtype=BF16)
    for j, b in enumerate(bs):
        pt = np.zeros((NS, V), dtype=np.float32)
        np.add.at(pt, (vidx[b].ravel(), np.repeat(np.arange(V), C)), vmask[b].ravel().astype(np.float32))
        poolt[:, j] = pt.reshape(2, 128, V).transpose(1, 0, 2).astype(BF16)
        invcnt[:, j] = 1.0 / np.maximum(vmask[b].sum(axis=1).astype(np.float32), 1.0)
        hsel[ht[b, :, 0], j, np.arange(R)] = BF16(1.0)
        tsel[ht[b, :, 1], j, np.arange(R)] = BF16(1.0)
        ehsel[dis_h[b], j, np.arange(R)] = BF16(1.0)
        etsel[dis_t[b], j, np.arange(R)] = BF16(1.0)

    return dict(
        sent=sent, gidx=gidx, w0m=w0mv, poolt=poolt, invcnt=invcnt,
        hsel=hsel, tsel=tsel, dist=dis_embed_b.T.copy(), ehsel=ehsel, etsel=etsel,
        w1=w1_p, w2=w2_p, b2t=b2_f,
    )


def run(trace=False, **inputs):
    global _NC_CACHE
    sentence_repr = np.asarray(inputs["sentence_repr"], dtype=np.float32)
    esi = np.asarray(inputs["entity_span_indices"]).astype(np.int64)
    vidx = np.asarray(inputs["vertex_indices"]).astype(np.int64)
    vmask = np.asarray(inputs["vertex_indices_mask"]).astype(np.int64)
    ht = np.asarray(inputs["head_tail_indices"]).astype(np.int64)
    dis_h = np.asarray(inputs["dis_h_2_t"]).astype(np.int64)
    dis_t = np.asarray(inputs["dis_t_2_h"]).astype(np.int64)
    dis_embed = np.asarray(inputs["dis_embed"], dtype=np.float32)
    w1 = np.asarray(inputs["W1"], dtype=np.float32)
    w2 = np.asarray(inputs["W2"], dtype=np.float32)
    b2 = np.asarray(inputs["b2"], dtype=np.float32)

    dis_embed_b = dis_embed.astype(BF16)
    # zero-pad W1 blocks to 20 uniform 128-row chunks, laid out [p, chunk, :]
    w1_pad = np.zeros((W1PAD, HID), dtype=BF16)
    dst = 0
    for r0, r1 in FEAT_BLOCKS:
        rows = r1 - r0
        nch = (rows + 127) // 128
        for i in range(nch):
            a = r0 + i * 128
            n = min(128, r1 - a)
            w1_pad[dst : dst + n] = w1[a : a + n].astype(BF16)
            dst += 128
    assert dst == W1PAD
    w1_p = np.ascontiguousarray(w1_pad.reshape(NKC, 128, HID).transpose(1, 0, 2))
    w2_p = np.ascontiguousarray(w2.astype(BF16).reshape(HID // 128, 128, REL).transpose(1, 0, 2))
    b2_f = b2.reshape(REL, 1).astype(np.float32)

    in_maps = [
        _prep_core(c, sentence_repr, esi, vidx, vmask, ht, dis_h, dis_t,
                   dis_embed_b, w1_p, w2_p, b2_f)
        for c in range(NCORES)
    ]

    if _NC_CACHE is None:
        _NC_CACHE = _build()

    res = bass_utils.run_bass_kernel_spmd(
        _NC_CACHE, in_maps, core_ids=list(range(NCORES)), trace=trace
    )

    out = np.empty((B, R, REL), dtype=np.float32)
    for c in range(NCORES):
        o = np.asarray(res.results[c]["outd"], dtype=np.float32)  # (128, NB, R)
        for j in range(NB):
            out[c * NB + j] = o[:REL, j].T
    return out, res


def kernel(**inputs):
    out, _ = run(**inputs)
    return out



# revision 5
# speedup vs baseline: 1.2281x; 1.2281x over previous
"""Trainium2 Bass kernel for nn_BiLSTM_M_61615600828569 (segment_reduce).

Full computation per batch:
  span_emb = masked-max-pool of token windows   (B,256,768)
  vertex_emb = masked-mean over coref spans     (B,128,768)
  head/tail  = vertex gather by relation        (B,512,768)
  feat = [head, eh, tail, et, head*tail]        (B,512,2344)
  out  = relu(feat @ W1) @ W2 + b2              (B,512,97)

Sharding: data-parallel over batch; 16 batches / 8 cores = 2 per core.

All index work happens on host, including the span-window gather itself:
for each span [s, e] the host stages 8 token rows
  [s, s+1^e, s+2^e, s+3^e, e-3|s, e-2|s, e-1|s, e]
(clamped duplicates instead of -inf masking -- every staged row is a
valid span member and their union covers [s, e] exactly for any width
0..7).  The device then reduces each span with a pure tensor_tensor max
pyramid -- no dma_gather, no gpsimd descriptor generation, no
scalar_tensor_tensor masking.

Device pipeline per batch: quad DMA chunks land -> DVE max pyramid per
128-span half -> PE: vertex pooling, V_emb^T, head/tail gather,
(V_emb @ W1) blocks, hidden accumulation (with the two distance-embed
tables stacked into a single K=40 matmul), output matmul; each batch's
output is DMA'd out as soon as it is ready.  All float math on device in
bf16 with fp32 PSUM accumulation, transposed layout (features on
partitions) so the final predict.T has the 97 classes on partitions for
a per-partition bias add.
"""
import numpy as np
import ml_dtypes
from contextlib import ExitStack

import concourse.bass as bass
import concourse.bacc as bacc
import concourse.tile as tile
from concourse import mybir
from concourse import bass_utils

BF16 = ml_dtypes.bfloat16

B, S, D = 16, 1024, 768
NS, MAXW = 256, 8
V, C = 128, 6
R = 512
REL, HID, DIS = 97, 384, 20

NCORES = 8
NB = B // NCORES          # batches per core = 2
NM = 6                    # 128-row d-chunks in D
NM3 = HID // 128          # hidden 128-row chunks = 3
NKA = 12                  # w1main chunks for a+c blocks
NKP = 6                   # w1main chunks for prod block
NKMAIN = NKA + NKP        # 18

# const blob (bf16) column layout: [poolt | hsel | tsel | w2]
CB_POOL = 0                       # [128, NB, 2, V]        NB*2*V   = 512
CB_HSEL = CB_POOL + NB * 2 * V    # [128, NB, R]           NB*R     = 1024
CB_TSEL = CB_HSEL + NB * R        # [128, NB, R]           NB*R     = 1024
CB_W2 = CB_TSEL + NB * R          # [128, 3, REL]          3*REL    = 291
CB_COLS = CB_W2 + NM3 * REL

# 40-partition blob (bf16): [disbd | distsel]
B40_DISBD = 0                     # [40, 40] block-diag dis_embed^T
B40_DSEL = 40                     # [40, NB, R] stacked ehsel/etsel
B40_COLS = B40_DSEL + NB * R


def _patch_drain_and_barrier():
    """Walrus rejects >1 explicit sync wait on a Drain (TPB_CTRL), but Tile's
    tail drain waits on every used proc sem at once. Emit one single-wait
    drain per proc instead; the final drain then needs no waits."""
    import concourse.tile as tile_mod
    from concourse.vector_clock import VectorClock, ScopedClock

    if getattr(tile_mod.TileContext, "_ant_drain_patched", False):
        return

    def _patched(self, tick_clock, wait_clock):
        full = tick_clock.global_clock
        n = len(full)
        engines = [self.nc.sync, self.nc.vector, self.nc.scalar,
                   self.nc.tensor, self.nc.gpsimd]
        for i, p in enumerate([q for q in range(n) if full[q] > 0]):
            vec = [full[q] if q == p else 0 for q in range(n)]
            d = engines[i % len(engines)].drain()
            wait_clock.add_sem_waits(d.ins, ScopedClock({None: VectorClock(vec)}))
        self.nc.sync.drain()
        self.nc.all_engine_barrier()
        popped = self.nc._tile_sem_poison_stack.pop()
        assert popped is self._sem_poison
        self.nc.clear_and_free_semaphores(list(self.sems.allocated().values()))
        self.nc.all_engine_barrier()

    tile_mod.TileContext._drain_and_barrier = _patched
    tile_mod.TileContext._ant_drain_patched = True


_patch_drain_and_barrier()

_NC_CACHE = None


def _build():
    """One-core program; SPMD-replicated across the 8 cores."""
    bf = mybir.dt.bfloat16
    f32 = mybir.dt.float32
    AF = mybir.ActivationFunctionType
    MAX = mybir.AluOpType.max

    nc = bacc.Bacc("TRN2", target_bir_lowering=False, debug=False, num_devices=1)

    # host-staged span windows: per (batch, span-half q, row-half) one
    # contiguous [128, 4*D] chunk; span i = q*128 + p.
    sq = [[[nc.dram_tensor(f"sq_{h}_{q}_{f}", (128, 4 * D), bf,
                           kind="ExternalInput")
            for f in range(2)] for q in range(2)] for h in range(NB)]
    cb16 = nc.dram_tensor("cb16", (128, CB_COLS), bf, kind="ExternalInput")
    b40 = nc.dram_tensor("b40", (40, B40_COLS), bf, kind="ExternalInput")
    w1m = nc.dram_tensor("w1m", (128, NKMAIN, HID), bf, kind="ExternalInput")
    w1bd = nc.dram_tensor("w1bd", (40, HID), bf, kind="ExternalInput")
    cb32 = nc.dram_tensor("cb32", (128, NB + 1), f32, kind="ExternalInput")
    outd = nc.dram_tensor("outd", (REL, NB, R), f32, kind="ExternalOutput")

    with tile.TileContext(nc) as tc, ExitStack() as ctx:
        consts = ctx.enter_context(tc.tile_pool(name="consts", bufs=1))
        work = ctx.enter_context(tc.tile_pool(name="work", bufs=1))
        perb = ctx.enter_context(tc.tile_pool(name="perb", bufs=2))
        psums = ctx.enter_context(tc.tile_pool(name="psums", bufs=1, space="PSUM"))

        def psum_tile(name, tag, bufs, shape=None):
            return psums.tile(shape or [128, R], mybir.dt.float32, space="PSUM",
                              tag=tag, bufs=bufs, name=name)

        # ---- input DMAs, in priority order, all issued on sync ----
        sq_t = [[[None] * 2 for _ in range(2)] for _ in range(NB)]
        for q in range(2):
            for f in range(2):
                t = work.tile([128, 4 * D], bf, name=f"sq_0_{q}_{f}",
                              tag=f"sq_0_{q}_{f}")
                nc.sync.dma_start(out=t[:], in_=sq[0][q][f].ap())
                sq_t[0][q][f] = t
        cb_t = consts.tile([128, CB_COLS], bf)
        nc.sync.dma_start(out=cb_t[:], in_=cb16.ap())
        b40_t = consts.tile([40, B40_COLS], bf)
        nc.sync.dma_start(out=b40_t[:], in_=b40.ap())
        cb32_t = consts.tile([128, NB + 1], f32)
        nc.sync.dma_start(out=cb32_t[:], in_=cb32.ap())
        w1bd_t = consts.tile([40, HID], bf)
        nc.sync.dma_start(out=w1bd_t[:], in_=w1bd.ap())
        w1_t = consts.tile([128, NKMAIN, HID], bf)
        nc.sync.dma_start(out=w1_t[:], in_=w1m.ap())
        for q in range(2):
            for f in range(2):
                t = work.tile([128, 4 * D], bf, name=f"sq_1_{q}_{f}",
                              tag=f"sq_1_{q}_{f}")
                nc.sync.dma_start(out=t[:], in_=sq[1][q][f].ap())
                sq_t[1][q][f] = t

        # const views
        poolt = cb_t[:, CB_POOL : CB_POOL + NB * 2 * V].rearrange(
            "p (b q v) -> p b q v", b=NB, q=2)
        hsel = cb_t[:, CB_HSEL : CB_HSEL + NB * R].rearrange(
            "p (b r) -> p b r", b=NB)
        tsel = cb_t[:, CB_TSEL : CB_TSEL + NB * R].rearrange(
            "p (b r) -> p b r", b=NB)
        w2sb = cb_t[:, CB_W2 : CB_W2 + NM3 * REL].rearrange(
            "p (k r) -> p k r", k=NM3)
        disbd = b40_t[:, B40_DISBD : B40_DISBD + 40]
        dsel = b40_t[:, B40_DSEL : B40_DSEL + NB * R].rearrange(
            "p (b r) -> p b r", b=NB)
        inv_t = cb32_t[:, 0:NB]
        b2_t = cb32_t[:, NB : NB + 1]

        # ---- EwS = blockdiag(disT, disT) @ [W1b; W1d]  (40, HID) ----
        ps_e = psum_tile("ps_e", "out", 1, shape=[40, HID])
        nc.tensor.matmul(ps_e[:], lhsT=disbd, rhs=w1bd_t[:], start=True,
                         stop=True)
        ews = consts.tile([40, HID], bf, name="ews")
        nc.scalar.activation(ews[:], ps_e[:], AF.Copy)

        # ---- per-batch: span max pyramid then compute ----
        sem_b = []  # sem_b[h][p, q, :] = span_emb[q*128 + p]
        for b in range(NB):
            sh = work.tile([128, 2, D], bf, name=f"sem_{b}", tag=f"sem_{b}")
            for q in range(2):
                t1 = work.tile([128, 4 * D], bf, name=f"t1_{b}_{q}", tag="t1",
                               bufs=2)
                nc.vector.tensor_tensor(out=t1[:], in0=sq_t[b][q][0][:],
                                        in1=sq_t[b][q][1][:], op=MAX)
                nc.vector.tensor_tensor(out=t1[:, 0 : 2 * D],
                                        in0=t1[:, 0 : 2 * D],
                                        in1=t1[:, 2 * D : 4 * D], op=MAX)
                nc.vector.tensor_tensor(out=sh[:, q, :], in0=t1[:, 0:D],
                                        in1=t1[:, D : 2 * D], op=MAX)
            sem_b.append(sh)

            # vertex pooling: V_emb = poolt^T @ span_emb (then * inv)
            ps_v = psums.tile([128, D], mybir.dt.float32, space="PSUM",
                              tag="ps_v", bufs=1, name="ps_v")
            for q in range(2):
                for n0, nsz in ((0, 512), (512, 256)):
                    nc.tensor.matmul(
                        ps_v[:, n0 : n0 + nsz],
                        lhsT=poolt[:, b, q, :],
                        rhs=sem_b[b][:, q, n0 : n0 + nsz],
                        start=(q == 0), stop=(q == 1),
                    )
            v_sb = perb.tile([V, D], bf, tag="v_sb")
            nc.scalar.activation(v_sb[:], ps_v[:], AF.Copy,
                                 scale=inv_t[:, b : b + 1])

            # V_emb^T chunks (unscaled; vw applies inv)
            vt_sb = perb.tile([128, NM, V], bf, tag="vt_sb")
            for m in range(NM):
                ps_vt = psum_tile("ps_vt", "sel", 3)
                for q in range(2):
                    nc.tensor.matmul(ps_vt[:, :V],
                                     lhsT=sem_b[b][:, q, m * 128 : (m + 1) * 128],
                                     rhs=poolt[:, b, q, :],
                                     start=(q == 0), stop=(q == 1))
                nc.any.tensor_copy(vt_sb[:, m, :], ps_vt[:, :V])

            # head/tail gather + product (prod in [d-part, m, r] layout)
            head_t = perb.tile([128, NM, R], bf, tag="head_t")
            tail_t = perb.tile([128, NM, R], bf, tag="tail_t")
            prod_t = perb.tile([128, NM, R], bf, tag="prod_t")
            for m in range(NM):
                ps_h = psum_tile("ps_h", "sel", 3)
                nc.tensor.matmul(ps_h[:], lhsT=v_sb[:, m * 128 : (m + 1) * 128],
                                 rhs=hsel[:, b, :], start=True, stop=True)
                nc.any.tensor_copy(head_t[:, m, :], ps_h[:])
                ps_t2 = psum_tile("ps_t2", "sel", 3)
                nc.tensor.matmul(ps_t2[:], lhsT=v_sb[:, m * 128 : (m + 1) * 128],
                                 rhs=tsel[:, b, :], start=True, stop=True)
                nc.any.tensor_copy(tail_t[:, m, :], ps_t2[:])
                nc.vector.tensor_tensor(out=prod_t[:, m, :], in0=head_t[:, m, :],
                                        in1=tail_t[:, m, :],
                                        op=mybir.AluOpType.mult)

            # Vw_a / Vw_c = (V_emb @ W1a|W1c) * inv
            vw_a = perb.tile([V, HID], bf, tag="vw_a")
            vw_c = perb.tile([V, HID], bf, tag="vw_c")
            ps_vw_a = psum_tile("ps_vw_a", "hid", 2)
            ps_vw_c = psum_tile("ps_vw_c", "hid", 2)
            for m in range(NM):
                nc.tensor.matmul(ps_vw_a[:, :HID], lhsT=vt_sb[:, m, :],
                                 rhs=w1_t[:, m, :],
                                 start=(m == 0), stop=(m == NM - 1))
                nc.tensor.matmul(ps_vw_c[:, :HID], lhsT=vt_sb[:, m, :],
                                 rhs=w1_t[:, NM + m, :],
                                 start=(m == 0), stop=(m == NM - 1))
            nc.scalar.activation(vw_a[:], ps_vw_a[:, :HID], AF.Copy,
                                 scale=inv_t[:, b : b + 1])
            nc.scalar.activation(vw_c[:], ps_vw_c[:, :HID], AF.Copy,
                                 scale=inv_t[:, b : b + 1])

            # hidden = relu( vw_a[h] + EwS-gather + vw_c[t] + W1p^T prod )
            hid_t = perb.tile([128, NM3, R], bf, tag="hid_t")
            for m3 in range(NM3):
                msl = slice(m3 * 128, (m3 + 1) * 128)
                ps_hid = psum_tile("ps_hid", "hid", 2)
                nc.tensor.matmul(ps_hid[:], lhsT=vw_a[:, msl], rhs=hsel[:, b, :],
                                 start=True, stop=False)
                nc.tensor.matmul(ps_hid[:], lhsT=vw_c[:, msl], rhs=tsel[:, b, :],
                                 start=False, stop=False)
                nc.tensor.matmul(ps_hid[:], lhsT=ews[:, msl], rhs=dsel[:, b, :],
                                 start=False, stop=False)
                for m in range(NM):
                    nc.tensor.matmul(ps_hid[:], lhsT=w1_t[:, NKA + m, msl],
                                     rhs=prod_t[:, m, :],
                                     start=False, stop=(m == NM - 1))
                nc.scalar.activation(hid_t[:, m3, :], ps_hid[:], AF.Relu)

            # out = W2^T @ hidden + b2, classes on partitions
            out_sb = perb.tile([128, R], f32, tag="out_sb")
            ps_o = psum_tile("ps_o", "out", 1)
            for kc in range(NM3):
                nc.tensor.matmul(ps_o[:REL, :], lhsT=w2sb[:, kc, :],
                                 rhs=hid_t[:, kc, :],
                                 start=(kc == 0), stop=(kc == NM3 - 1))
            nc.scalar.activation(out_sb[:REL, :], ps_o[:REL, :], AF.Identity,
                                 bias=b2_t[:REL, 0:1])
            nc.sync.dma_start(out=outd.ap()[:, b, :], in_=out_sb[:REL, :])

    nc.compile()
    return nc


def _prep_core(c, sentence_repr, esi, vidx, vmask, ht, dis_h, dis_t,
               dis_embed, w1m_p, w1bd_p, w2_p, b2):
    """Build the per-core input map for batches [c*NB, c*NB+NB)."""
    bs = range(c * NB, c * NB + NB)
    inputs = {}

    poolt = np.zeros((128, NB, 2, V), dtype=BF16)
    hsel = np.zeros((V, NB, R), dtype=BF16)
    tsel = np.zeros((V, NB, R), dtype=BF16)
    dsel = np.zeros((40, NB, R), dtype=BF16)
    invcnt = np.zeros((V, NB), dtype=np.float32)
    rr = np.arange(R)

    for j, b in enumerate(bs):
        # staged span windows: 8 valid (clamped-duplicate) rows per span
        s = esi[b, :, 0]
        e = esi[b, :, 1]
        rows = np.stack([s,
                         np.minimum(s + 1, e),
                         np.minimum(s + 2, e),
                         np.minimum(s + 3, e),
                         np.maximum(e - 3, s),
                         np.maximum(e - 2, s),
                         np.maximum(e - 1, s),
                         e], axis=1)                     # (NS, 8)
        gath = sentence_repr[b][rows]                    # (NS, 8, D) f32
        gath = gath.reshape(2, 128, 2, 4 * D).astype(BF16)   # q, p, f, :
        for q in range(2):
            for f in range(2):
                inputs[f"sq_{j}_{q}_{f}"] = np.ascontiguousarray(gath[q, :, f])

        pt = np.zeros((NS, V), dtype=np.float32)
        np.add.at(pt, (vidx[b].ravel(), np.repeat(np.arange(V), C)),
                  vmask[b].ravel().astype(np.float32))
        poolt[:, j] = pt.reshape(2, 128, V).transpose(1, 0, 2).astype(BF16)
        invcnt[:, j] = 1.0 / np.maximum(vmask[b].sum(axis=1).astype(np.float32), 1.0)
        hsel[ht[b, :, 0], j, rr] = BF16(1.0)
        tsel[ht[b, :, 1], j, rr] = BF16(1.0)
        dsel[dis_h[b], j, rr] = BF16(1.0)
        dsel[20 + dis_t[b], j, rr] = BF16(1.0)

    cb16 = np.zeros((128, CB_COLS), dtype=BF16)
    cb16[:, CB_POOL : CB_POOL + NB * 2 * V] = poolt.reshape(128, -1)
    cb16[:V, CB_HSEL : CB_HSEL + NB * R] = hsel.reshape(V, -1)
    cb16[:V, CB_TSEL : CB_TSEL + NB * R] = tsel.reshape(V, -1)
    cb16[:, CB_W2 : CB_W2 + NM3 * REL] = w2_p.reshape(128, -1)
    inputs["cb16"] = cb16

    b40 = np.zeros((40, B40_COLS), dtype=BF16)
    b40[:20, 0:20] = dis_embed.T.astype(BF16)
    b40[20:40, 20:40] = dis_embed.T.astype(BF16)
    b40[:, B40_DSEL : B40_DSEL + NB * R] = dsel.reshape(40, -1)
    inputs["b40"] = b40

    cb32 = np.zeros((128, NB + 1), dtype=np.float32)
    cb32[:V, 0:NB] = invcnt
    cb32[:REL, NB] = b2
    inputs["cb32"] = cb32

    inputs["w1m"] = w1m_p
    inputs["w1bd"] = w1bd_p
    return inputs


def run(trace=False, **inputs):
    global _NC_CACHE
    sentence_repr = np.asarray(inputs["sentence_repr"], dtype=np.float32)
    esi = np.asarray(inputs["entity_span_indices"]).astype(np.int64)
    vidx = np.asarray(inputs["vertex_indices"]).astype(np.int64)
    vmask = np.asarray(inputs["vertex_indices_mask"]).astype(np.int64)
    ht = np.asarray(inputs["head_tail_indices"]).astype(np.int64)
    dis_h = np.asarray(inputs["dis_h_2_t"]).astype(np.int64)
    dis_t = np.asarray(inputs["dis_t_2_h"]).astype(np.int64)
    dis_embed = np.asarray(inputs["dis_embed"], dtype=np.float32)
    w1 = np.asarray(inputs["W1"], dtype=np.float32)
    w2 = np.asarray(inputs["W2"], dtype=np.float32)
    b2 = np.asarray(inputs["b2"], dtype=np.float32)

    # W1 row blocks: a (0:768) -> chunks 0-5, c (788:1556) -> 6-11,
    # p (1576:2344) -> 12-17; b (768:788) + d (1556:1576) -> w1bd (40, HID).
    w1m_rows = np.concatenate([w1[0:768], w1[788:1556], w1[1576:2344]])
    w1m_p = np.ascontiguousarray(
        w1m_rows.astype(BF16).reshape(NKMAIN, 128, HID).transpose(1, 0, 2))
    w1bd_p = np.ascontiguousarray(
        np.concatenate([w1[768:788], w1[1556:1576]]).astype(BF16))
    w2_p = np.ascontiguousarray(
        w2.astype(BF16).reshape(NM3, 128, REL).transpose(1, 0, 2))

    in_maps = [
        _prep_core(c, sentence_repr, esi, vidx, vmask, ht, dis_h, dis_t,
                   dis_embed, w1m_p, w1bd_p, w2_p, b2)
        for c in range(NCORES)
    ]

    if _NC_CACHE is None:
        _NC_CACHE = _build()

    res = bass_utils.run_bass_kernel_spmd(
        _NC_CACHE, in_maps, core_ids=list(range(NCORES)), trace=trace
    )

    out = np.empty((B, R, REL), dtype=np.float32)
    for c in range(NCORES):
        o = np.asarray(res.results[c]["outd"], dtype=np.float32)  # (REL, NB, R)
        for j in range(NB):
            out[c * NB + j] = o[:, j].T
    return out, res


def kernel(**inputs):
    out, _ = run(**inputs)
    return out


# revision 15
# speedup vs baseline: 1.3522x; 1.1011x over previous
"""Trainium2 Bass kernel for nn_BiLSTM_M_61615600828569 (segment_reduce).

Full computation per batch:
  span_emb = masked-max-pool of token windows   (B,256,768)
  vertex_emb = masked-mean over coref spans     (B,128,768)
  head/tail  = vertex gather by relation        (B,512,768)
  feat = [head, eh, tail, et, head*tail]        (B,512,2344)
  out  = relu(feat @ W1) @ W2 + b2              (B,512,97)

Sharding: data-parallel over batch; 16 batches / 8 cores = 2 per core.

All index work happens on host, including the span-window gather itself:
for each span [s, e] the host stages 8 token rows
  [s, s+1^e, s+2^e, s+3^e, e-3|s, e-2|s, e-1|s, e]
(clamped duplicates instead of -inf masking -- every staged row is a
valid span member and their union covers [s, e] exactly for any width
0..7).  The device then reduces each span with a pure tensor_tensor max
pyramid -- no dma_gather, no gpsimd descriptor generation, no
scalar_tensor_tensor masking.

Device pipeline per batch: quad DMA chunks land -> DVE max pyramid per
128-span half -> PE: vertex pooling, V_emb^T, head/tail gather,
(V_emb @ W1) blocks, hidden accumulation (with the two distance-embed
tables stacked into a single K=40 matmul), output matmul; each batch's
output is DMA'd out as soon as it is ready.  All float math on device in
bf16 with fp32 PSUM accumulation, transposed layout (features on
partitions) so the final predict.T has the 97 classes on partitions for
a per-partition bias add.
"""
import numpy as np
import ml_dtypes
from contextlib import ExitStack

import concourse.bass as bass
import concourse.bacc as bacc
import concourse.tile as tile
from concourse import mybir
from concourse import bass_utils

BF16 = ml_dtypes.bfloat16

B, S, D = 16, 1024, 768
NS, MAXW = 256, 8
V, C = 128, 6
R = 512
REL, HID, DIS = 97, 384, 20

NCORES = 8
NB = B // NCORES          # batches per core = 2
NM = 6                    # 128-row d-chunks in D
NM3 = HID // 128          # hidden 128-row chunks = 3
NKA = 12                  # w1main chunks for a+c blocks
NKP = 6                   # w1main chunks for prod block
NKMAIN = NKA + NKP        # 18

# early const blob (bf16, lands first): [poolt | disbd | distsel | w1bd]
# rows 0..39 carry the 40-partition dis tables/weights in their own columns.
CA_POOL = 0                       # [128, NB, 2, V]        NB*2*V   = 512
CA_DISBD = CA_POOL + NB * 2 * V   # [40, 40] block-diag dis_embed^T
CA_DSEL = CA_DISBD + 40           # [40, NB, R] stacked ehsel/etsel
CA_W1BD = CA_DSEL + NB * R        # [40, HID] stacked W1b/W1d rows
CA_COLS = CA_W1BD + HID

# late const blob (bf16): [hsel | tsel | w2]
CB_HSEL = 0                       # [128, NB, R]           NB*R     = 1024
CB_TSEL = CB_HSEL + NB * R        # [128, NB, R]           NB*R     = 1024
CB_W2 = CB_TSEL + NB * R          # [128, 3, REL]          3*REL    = 291
CB_COLS = CB_W2 + NM3 * REL


def _patch_drain_and_barrier():
    """Walrus rejects >1 explicit sync wait on a Drain (TPB_CTRL), but Tile's
    tail drain waits on every used proc sem at once. Emit one single-wait
    drain per proc instead; the final drain then needs no waits."""
    import concourse.tile as tile_mod
    from concourse.vector_clock import VectorClock, ScopedClock

    if getattr(tile_mod.TileContext, "_ant_drain_patched", False):
        return

    def _patched(self, tick_clock, wait_clock):
        full = tick_clock.global_clock
        n = len(full)
        engines = [self.nc.sync, self.nc.vector, self.nc.scalar,
                   self.nc.tensor, self.nc.gpsimd]
        for i, p in enumerate([q for q in range(n) if full[q] > 0]):
            vec = [full[q] if q == p else 0 for q in range(n)]
            d = engines[i % len(engines)].drain()
            wait_clock.add_sem_waits(d.ins, ScopedClock({None: VectorClock(vec)}))
        self.nc.sync.drain()
        self.nc.all_engine_barrier()
        popped = self.nc._tile_sem_poison_stack.pop()
        assert popped is self._sem_poison
        self.nc.clear_and_free_semaphores(list(self.sems.allocated().values()))
        self.nc.all_engine_barrier()

    tile_mod.TileContext._drain_and_barrier = _patched
    tile_mod.TileContext._ant_drain_patched = True


_patch_drain_and_barrier()

_NC_CACHE = None


def _build():
    """One-core program; SPMD-replicated across the 8 cores."""
    bf = mybir.dt.bfloat16
    f32 = mybir.dt.float32
    AF = mybir.ActivationFunctionType
    MAX = mybir.AluOpType.max

    nc = bacc.Bacc("TRN2", target_bir_lowering=False, debug=False, num_devices=1)

    # host-staged span windows: per (batch, span-half q, row-half) one
    # contiguous [128, 4*D] chunk; span i = q*128 + p.
    sq = [[[nc.dram_tensor(f"sq_{h}_{q}_{f}", (128, 4 * D), bf,
                           kind="ExternalInput")
            for f in range(2)] for q in range(2)] for h in range(NB)]
    cba = nc.dram_tensor("cba", (128, CA_COLS), bf, kind="ExternalInput")
    cbb = nc.dram_tensor("cbb", (128, CB_COLS), bf, kind="ExternalInput")
    w1ac = nc.dram_tensor("w1ac", (128, NKA, HID), bf, kind="ExternalInput")
    w1p = nc.dram_tensor("w1p", (128, NKP, HID), bf, kind="ExternalInput")
    cb32 = nc.dram_tensor("cb32", (128, NB + 1), f32, kind="ExternalInput")
    outd = [nc.dram_tensor(f"outd{b}", (128, R), f32, kind="ExternalOutput")
            for b in range(NB)]

    with tile.TileContext(nc) as tc, ExitStack() as ctx:
        consts = ctx.enter_context(tc.tile_pool(name="consts", bufs=1))
        work = ctx.enter_context(tc.tile_pool(name="work", bufs=1))
        perb = ctx.enter_context(tc.tile_pool(name="perb", bufs=2))
        psums = ctx.enter_context(tc.tile_pool(name="psums", bufs=1, space="PSUM"))

        def psum_tile(name, tag, bufs, shape=None):
            return psums.tile(shape or [128, R], mybir.dt.float32, space="PSUM",
                              tag=tag, bufs=bufs, name=name)

        # ---- input DMAs, in priority order, all issued on sync ----
        cba_t = consts.tile([128, CA_COLS], bf)
        nc.sync.dma_start(out=cba_t[:], in_=cba.ap())
        cb32_t = consts.tile([128, NB + 1], f32)
        nc.sync.dma_start(out=cb32_t[:], in_=cb32.ap())
        sq_t = [[[None] * 2 for _ in range(2)] for _ in range(NB)]
        for q in range(2):
            for f in range(2):
                t = work.tile([128, 4 * D], bf, name=f"sq_0_{q}_{f}",
                              tag=f"sq_0_{q}_{f}")
                nc.sync.dma_start(out=t[:], in_=sq[0][q][f].ap())
                sq_t[0][q][f] = t
        cb_t = consts.tile([128, CB_COLS], bf)
        nc.sync.dma_start(out=cb_t[:], in_=cbb.ap())
        w1_t = consts.tile([128, NKA, HID], bf)
        nc.sync.dma_start(out=w1_t[:], in_=w1ac.ap())
        w1p_t = consts.tile([128, NKP, HID], bf)
        nc.sync.dma_start(out=w1p_t[:], in_=w1p.ap())
        for q in range(2):
            for f in range(2):
                t = work.tile([128, 4 * D], bf, name=f"sq_1_{q}_{f}",
                              tag=f"sq_1_{q}_{f}")
                nc.sync.dma_start(out=t[:], in_=sq[1][q][f].ap())
                sq_t[1][q][f] = t

        # const views
        poolt = cba_t[:, CA_POOL : CA_POOL + NB * 2 * V].rearrange(
            "p (b q v) -> p b q v", b=NB, q=2)
        disbd = cba_t[:40, CA_DISBD : CA_DISBD + 40]
        dsel = cba_t[:40, CA_DSEL : CA_DSEL + NB * R].rearrange(
            "p (b r) -> p b r", b=NB)
        w1bd_t = cba_t[:40, CA_W1BD : CA_W1BD + HID]
        hsel = cb_t[:, CB_HSEL : CB_HSEL + NB * R].rearrange(
            "p (b r) -> p b r", b=NB)
        tsel = cb_t[:, CB_TSEL : CB_TSEL + NB * R].rearrange(
            "p (b r) -> p b r", b=NB)
        w2sb = cb_t[:, CB_W2 : CB_W2 + NM3 * REL].rearrange(
            "p (k r) -> p k r", k=NM3)
        inv_t = cb32_t[:, 0:NB]
        b2_t = cb32_t[:, NB : NB + 1]

        # ---- per-batch: span max pyramid then compute ----
        sem_b = []  # sem_b[h][p, q, :] = span_emb[q*128 + p]
        for b in range(NB):
            sh = work.tile([128, 2, D], bf, name=f"sem_{b}", tag=f"sem_{b}")
            for q in range(2):
                t1 = work.tile([128, 4 * D], bf, name=f"t1_{b}_{q}", tag="t1",
                               bufs=2)
                nc.vector.tensor_tensor(out=t1[:], in0=sq_t[b][q][0][:],
                                        in1=sq_t[b][q][1][:], op=MAX)
                nc.vector.tensor_tensor(out=t1[:, 0 : 2 * D],
                                        in0=t1[:, 0 : 2 * D],
                                        in1=t1[:, 2 * D : 4 * D], op=MAX)
                nc.vector.tensor_tensor(out=sh[:, q, :], in0=t1[:, 0:D],
                                        in1=t1[:, D : 2 * D], op=MAX)
            sem_b.append(sh)

            # vertex pooling: V_emb = poolt^T @ span_emb (then * inv)
            ps_v = psums.tile([128, D], mybir.dt.float32, space="PSUM",
                              tag="ps_v", bufs=1, name="ps_v")
            for q in range(2):
                for n0, nsz in ((0, 512), (512, 256)):
                    nc.tensor.matmul(
                        ps_v[:, n0 : n0 + nsz],
                        lhsT=poolt[:, b, q, :],
                        rhs=sem_b[b][:, q, n0 : n0 + nsz],
                        start=(q == 0), stop=(q == 1),
                    )
            v_sb = perb.tile([V, D], bf, tag="v_sb")
            nc.scalar.activation(v_sb[:], ps_v[:], AF.Copy,
                                 scale=inv_t[:, b : b + 1])

            # V_emb^T chunks (unscaled; vw applies inv)
            vt_sb = perb.tile([128, NM, V], bf, tag="vt_sb")
            for m in range(NM):
                ps_vt = psum_tile("ps_vt", "sel", 3)
                for q in range(2):
                    nc.tensor.matmul(ps_vt[:, :V],
                                     lhsT=sem_b[b][:, q, m * 128 : (m + 1) * 128],
                                     rhs=poolt[:, b, q, :],
                                     start=(q == 0), stop=(q == 1))
                nc.any.tensor_copy(vt_sb[:, m, :], ps_vt[:, :V])

            # head/tail gather + product (prod in [d-part, m, r] layout)
            head_t = perb.tile([128, NM, R], bf, tag="head_t")
            tail_t = perb.tile([128, NM, R], bf, tag="tail_t")
            prod_t = perb.tile([128, NM, R], bf, tag="prod_t")
            for m in range(NM):
                ps_h = psum_tile("ps_h", "sel", 3)
                nc.tensor.matmul(ps_h[:], lhsT=v_sb[:, m * 128 : (m + 1) * 128],
                                 rhs=hsel[:, b, :], start=True, stop=True)
                nc.any.tensor_copy(head_t[:, m, :], ps_h[:])
                ps_t2 = psum_tile("ps_t2", "sel", 3)
                nc.tensor.matmul(ps_t2[:], lhsT=v_sb[:, m * 128 : (m + 1) * 128],
                                 rhs=tsel[:, b, :], start=True, stop=True)
                nc.any.tensor_copy(tail_t[:, m, :], ps_t2[:])
                nc.vector.tensor_tensor(out=prod_t[:, m, :], in0=head_t[:, m, :],
                                        in1=tail_t[:, m, :],
                                        op=mybir.AluOpType.mult)

            if b == 0:
                # EwS = blockdiag(disT, disT) @ [W1b; W1d]  (40, HID)
                ps_e = psum_tile("ps_e", "out", 1, shape=[40, HID])
                nc.tensor.matmul(ps_e[:], lhsT=disbd, rhs=w1bd_t,
                                 start=True, stop=True)
                ews = consts.tile([40, HID], bf, name="ews")
                nc.scalar.activation(ews[:], ps_e[:], AF.Copy)

            # Vw_a / Vw_c = (V_emb @ W1a|W1c) * inv
            vw_a = perb.tile([V, HID], bf, tag="vw_a")
            vw_c = perb.tile([V, HID], bf, tag="vw_c")
            ps_vw_a = psum_tile("ps_vw_a", "hid", 2)
            ps_vw_c = psum_tile("ps_vw_c", "hid", 2)
            for m in range(NM):
                nc.tensor.matmul(ps_vw_a[:, :HID], lhsT=vt_sb[:, m, :],
                                 rhs=w1_t[:, m, :],
                                 start=(m == 0), stop=(m == NM - 1))
                nc.tensor.matmul(ps_vw_c[:, :HID], lhsT=vt_sb[:, m, :],
                                 rhs=w1_t[:, NM + m, :],
                                 start=(m == 0), stop=(m == NM - 1))
            nc.scalar.activation(vw_a[:], ps_vw_a[:, :HID], AF.Copy,
                                 scale=inv_t[:, b : b + 1])
            nc.scalar.activation(vw_c[:], ps_vw_c[:, :HID], AF.Copy,
                                 scale=inv_t[:, b : b + 1])

            # hidden = relu( vw_a[h] + EwS-gather + vw_c[t] + W1p^T prod )
            hid_t = perb.tile([128, NM3, R], bf, tag="hid_t")
            for m3 in range(NM3):
                msl = slice(m3 * 128, (m3 + 1) * 128)
                ps_hid = psum_tile("ps_hid", "hid", 2)
                nc.tensor.matmul(ps_hid[:], lhsT=vw_a[:, msl], rhs=hsel[:, b, :],
                                 start=True, stop=False)
                nc.tensor.matmul(ps_hid[:], lhsT=vw_c[:, msl], rhs=tsel[:, b, :],
                                 start=False, stop=False)
                nc.tensor.matmul(ps_hid[:], lhsT=ews[:, msl], rhs=dsel[:, b, :],
                                 start=False, stop=False)
                for m in range(NM):
                    nc.tensor.matmul(ps_hid[:], lhsT=w1p_t[:, m, msl],
                                     rhs=prod_t[:, m, :],
                                     start=False, stop=(m == NM - 1))
                nc.scalar.activation(hid_t[:, m3, :], ps_hid[:], AF.Relu)

            # out = W2^T @ hidden + b2, classes on partitions
            out_sb = perb.tile([128, R], f32, tag="out_sb")
            ps_o = psum_tile("ps_o", "out", 1)
            for kc in range(NM3):
                nc.tensor.matmul(ps_o[:REL, :], lhsT=w2sb[:, kc, :],
                                 rhs=hid_t[:, kc, :],
                                 start=(kc == 0), stop=(kc == NM3 - 1))
            nc.scalar.activation(out_sb[:REL, :], ps_o[:REL, :], AF.Identity,
                                 bias=b2_t[:REL, 0:1])
            nc.sync.dma_start(out=outd[b].ap(), in_=out_sb[:])

    nc.compile()
    return nc


def _prep_core(c, sentence_repr, esi, vidx, vmask, ht, dis_h, dis_t,
               dis_embed, w1ac_p, w1p_p, w1bd_p, w2_p, b2):
    """Build the per-core input map for batches [c*NB, c*NB+NB)."""
    bs = range(c * NB, c * NB + NB)
    inputs = {"w1ac": w1ac_p, "w1p": w1p_p}

    poolt = np.zeros((128, NB, 2, V), dtype=BF16)
    hsel = np.zeros((V, NB, R), dtype=BF16)
    tsel = np.zeros((V, NB, R), dtype=BF16)
    dsel = np.zeros((40, NB, R), dtype=BF16)
    invcnt = np.zeros((V, NB), dtype=np.float32)
    rr = np.arange(R)

    for j, b in enumerate(bs):
        # staged span windows: 8 valid (clamped-duplicate) rows per span
        s = esi[b, :, 0]
        e = esi[b, :, 1]
        rows = np.stack([s,
                         np.minimum(s + 1, e),
                         np.minimum(s + 2, e),
                         np.minimum(s + 3, e),
                         np.maximum(e - 3, s),
                         np.maximum(e - 2, s),
                         np.maximum(e - 1, s),
                         e], axis=1)                     # (NS, 8)
        gath = sentence_repr[b][rows]                    # (NS, 8, D) f32
        gath = gath.reshape(2, 128, 2, 4 * D).astype(BF16)   # q, p, f, :
        for q in range(2):
            for f in range(2):
                inputs[f"sq_{j}_{q}_{f}"] = np.ascontiguousarray(gath[q, :, f])

        pt = np.zeros((NS, V), dtype=np.float32)
        np.add.at(pt, (vidx[b].ravel(), np.repeat(np.arange(V), C)),
                  vmask[b].ravel().astype(np.float32))
        poolt[:, j] = pt.reshape(2, 128, V).transpose(1, 0, 2).astype(BF16)
        invcnt[:, j] = 1.0 / np.maximum(vmask[b].sum(axis=1).astype(np.float32), 1.0)
        hsel[ht[b, :, 0], j, rr] = BF16(1.0)
        tsel[ht[b, :, 1], j, rr] = BF16(1.0)
        dsel[dis_h[b], j, rr] = BF16(1.0)
        dsel[20 + dis_t[b], j, rr] = BF16(1.0)

    cba = np.zeros((128, CA_COLS), dtype=BF16)
    cba[:, CA_POOL : CA_POOL + NB * 2 * V] = poolt.reshape(128, -1)
    cba[:20, CA_DISBD : CA_DISBD + 20] = dis_embed.T.astype(BF16)
    cba[20:40, CA_DISBD + 20 : CA_DISBD + 40] = dis_embed.T.astype(BF16)
    cba[:40, CA_DSEL : CA_DSEL + NB * R] = dsel.reshape(40, -1)
    cba[:40, CA_W1BD : CA_W1BD + HID] = w1bd_p
    inputs["cba"] = cba

    cbb = np.zeros((128, CB_COLS), dtype=BF16)
    cbb[:V, CB_HSEL : CB_HSEL + NB * R] = hsel.reshape(V, -1)
    cbb[:V, CB_TSEL : CB_TSEL + NB * R] = tsel.reshape(V, -1)
    cbb[:, CB_W2 : CB_W2 + NM3 * REL] = w2_p.reshape(128, -1)
    inputs["cbb"] = cbb

    cb32 = np.zeros((128, NB + 1), dtype=np.float32)
    cb32[:V, 0:NB] = invcnt
    cb32[:REL, NB] = b2
    inputs["cb32"] = cb32
    return inputs


def run(trace=False, **inputs):
    global _NC_CACHE
    sentence_repr = np.asarray(inputs["sentence_repr"], dtype=np.float32)
    esi = np.asarray(inputs["entity_span_indices"]).astype(np.int64)
    vidx = np.asarray(inputs["vertex_indices"]).astype(np.int64)
    vmask = np.asarray(inputs["vertex_indices_mask"]).astype(np.int64)
    ht = np.asarray(inputs["head_tail_indices"]).astype(np.int64)
    dis_h = np.asarray(inputs["dis_h_2_t"]).astype(np.int64)
    dis_t = np.asarray(inputs["dis_t_2_h"]).astype(np.int64)
    dis_embed = np.asarray(inputs["dis_embed"], dtype=np.float32)
    w1 = np.asarray(inputs["W1"], dtype=np.float32)
    w2 = np.asarray(inputs["W2"], dtype=np.float32)
    b2 = np.asarray(inputs["b2"], dtype=np.float32)

    # W1 row blocks: a (0:768) -> w1ac chunks 0-5, c (788:1556) -> 6-11,
    # p (1576:2344) -> w1p chunks 0-5; b (768:788) + d (1556:1576) -> w1bd.
    w1ac_rows = np.concatenate([w1[0:768], w1[788:1556]])
    w1ac_p = np.ascontiguousarray(
        w1ac_rows.astype(BF16).reshape(NKA, 128, HID).transpose(1, 0, 2))
    w1p_p = np.ascontiguousarray(
        w1[1576:2344].astype(BF16).reshape(NKP, 128, HID).transpose(1, 0, 2))
    w1bd_p = np.ascontiguousarray(
        np.concatenate([w1[768:788], w1[1556:1576]]).astype(BF16))
    w2_p = np.ascontiguousarray(
        w2.astype(BF16).reshape(NM3, 128, REL).transpose(1, 0, 2))

    in_maps = [
        _prep_core(c, sentence_repr, esi, vidx, vmask, ht, dis_h, dis_t,
                   dis_embed, w1ac_p, w1p_p, w1bd_p, w2_p, b2)
        for c in range(NCORES)
    ]

    if _NC_CACHE is None:
        _NC_CACHE = _build()

    res = bass_utils.run_bass_kernel_spmd(
        _NC_CACHE, in_maps, core_ids=list(range(NCORES)), trace=trace
    )

    out = np.empty((B, R, REL), dtype=np.float32)
    for c in range(NCORES):
        for j in range(NB):
            o = np.asarray(res.results[c][f"outd{j}"], dtype=np.float32)
            out[c * NB + j] = o[:REL].T
    return out, res


def kernel(**inputs):
    out, _ = run(**inputs)
    return out
